# revision 25
# baseline (speedup 1.0000x reference)
"""KNNDistanceLoss Trainium2 Bass kernel (v2).

loss = pearson_loss(2000-sample distance matrices) + 0.5 * local_loss
local_loss = mean over (i, 85 coord-NN of i) of (d_emb - d_coord)^2
             * exp(-gamma * d_coord)

Design (8 cores, each owns N/8 = 1536 query rows; keys replicated):
  - All norms folded into the augmented matmul, so PSUM holds -d^2
    directly (coords + |k|^2 + |q|^2 rows, hi/lo bf16 splits; exact to
    ~2^-16 rel).
  - ACT copies bf16(-d^2_coord) into the HIGH u16 halves of an f32
    "packed" array whose LOW halves hold a column-index constant: each
    packed f32 sorts by distance value and carries its column index.
  - DVE extraction: per-256-column max8 (top-8, prob of losing a true
    top-86 neighbour ~1e-4 per row-half, error per event ~1e-4 of the
    loss) -> 384 candidates; 11 rounds max8+match_replace -> top-88
    packed values = values + column indices. No max_index, no masks.
  - Sparse term phase: decode (bit ops), sqrt on [128,88] tiles,
    exp via DVE polynomial (keeps ACT on the sqrt table set forever),
    GPSIMD indirect_copy gathers pred at the selected columns
    (16-partition-wrapped; each row's own values land at stride-16
    diagonal slots, selected by a static mask), fused
    tensor_tensor_reduce accumulates (pred-true)^2 * w.
  - pred = sqrt(-d^2_emb) is produced straight from PSUM by the ACT
    sqrt move (the move IS the sqrt).
Self distance ~0 ranks first and contributes a zero term, matching the
reference's topk(k+1) drop-self exactly.
"""

import base64
import zlib
from contextlib import ExitStack

import numpy as np
import ml_dtypes

import concourse.bass as bass
import concourse.bacc as bacc
import concourse.mybir as mybir
import concourse.tile as tile
from concourse import library_config
from concourse.bass_utils import run_bass_kernel_spmd

F32 = mybir.dt.float32
BF16 = mybir.dt.bfloat16
U16 = mybir.dt.uint16
U32 = mybir.dt.uint32
AF = mybir.ActivationFunctionType
ALU = mybir.AluOpType

BF = ml_dtypes.bfloat16

N, D, C = 12288, 64, 3
KNN = 85
SEL = KNN + 1  # 86 = self + 85 nearest
NEX = 88      # extracted per row (11 rounds of max8)
GAMMA = 0.5
SAMPLE = 2000
NCORES = 8
ROWS = N // NCORES   # 1536
NB = ROWS // 128     # 12
NCH = N // 2048      # 6 psum-groups per block (coord)
NEH = N // 1024      # 12 psum-groups per block (emb)

NEG_BIG = -1.0e30

# exp(-GAMMA*d) on [0, 10] via degree-8 power-series fit (abs err ~2e-5)
_dg = np.linspace(0.0, 10.0, 4001)
_cheb = np.polynomial.chebyshev.Chebyshev.fit(_dg, np.exp(-GAMMA * _dg), 8)
EXP_COEF = _cheb.convert(kind=np.polynomial.Polynomial).coef  # c0..c8

# jax.random.permutation(jax.random.key(42), 12288)[:2000] -- fixed sample.
IDX_B64 = "eNoFwQdg1IQCANDcXcYluVz25VZyl1zGJZdcEIXC56NsLBakLAvSsgWRKSC7yBCQIXujoEwLZUjtB0FBQGTPyqgs2QKytAgC8t9rhYyFNykYeZct9Xwa0ZPRWI1kD2kX7yHexquwdtpgN58cTQ41Fpuj9LjnmZRGQ66gzfeJ+gt+hVwgtify4frkcP47Y1MyD++Dp/yzYtsFUpjLRYPL5G+ZRfH7RA5UL9hS5oVJqffYu5EG8C6BTe/lroBxe3ESkNqkDwmk0jJ5K/owK7oHuV5qP2khOdboIJXjfcL12I2BF0ROZqaVBT5lV+LD1E7+7xU5dhHel2wr8c4i/TX6hlSV1MKNiLxQT08dqbu1EqprIfYYJxSdG7kF/GGfkBtg4+zrZnHqDnuaCVG3qKHoYHUitDA9PdZYe8XTG3gd6OpFgzWAHzM3UiPd0cIytix71b5mdPKsMz5L7fBfym5XD9ALUqvtvoFPYiuiTzzj0meBU9YEf54/q7dnlwGqQlGs/Y3gj32ObEsWA3v1A1JzJ2s+kSHrgDqGQqQdQCfIz+alHjoEnU1tz85g91B+upf5UE0S1cCxYnH8kGcJ/zy5kulCTonXDP6VNcwSZk3mebA1B5qvcQv5qwkdZN2W1DUxxgN6F6RH6AzTLbjf68/UticCl0PXoOpg10SFZ4u2yAupHnqQhqD3mZPKAvRQdl/iKTRA/hN5BX4ZNuHR1mJ6I/w80iE20fREe6rrxcbmc3IYvQU/CXYWmzlX03M9cfKaWQ//U2qXaqDrbA/nBXNIbEvkSb3CU/UJXMzfwLlu1MFvseeF98KVyICYz2HNof4W9nd4ezHpXkc3J6/BcpTOnMa6R65o7eGmqCdxK3JasaLZTE0+xUyF7pun4GqEpH7NBdwCbBq/O3nXHZRdR/f2TpUGSGP9uvy5NTW+OpKrS77jyh60InkOGe8WJm5YzSE9lmZ/ImpEsp5S82MJ89xXJwcroZ4ozrNJLBhhX6W28zvSI/RCa0TigtyTwGwZTCdf4PuJHPxLd467RulplzGD4guc77njIUZZbw6KnbTC2YhSI9sh8lwuAnZD85O3qRD5LXnULgiNwK4DR7J9oxOZ5XGMDmQ3iL/FG2ffMAWxfuxRtHfoGfGCa8btIE8nshHSuBtfZf2PHKo94IBAT+s896O5FhD5y/Accm94HbJA3xP+J1YQ3aGctDcibwk3fbnA5/iL1LvWf9AF7C38ReZl8kF2Dz+LWpXoI3isPtZ1zaf8g7X3pJKrmHxfB3ID2A2+ZO2NrGdIchzXP7Qb3xR4ht+0RjM/EyPjc2KV0bUSmN2gve5tKLeKHYDGwR0ZLj1LHG81ie+S9ioLTRQbCf9CFyR+Nw7qY1NToJGe8sBd77IUCg62X4k3yPwEDI9+k30rVAkeYcv9/1qvBkc6pdFjbGWsjm8eeBEYgWSxLfAH3rpQ50hx6At2D4N4G5Mmc9oZ6T+lFnAO/h68HPhUv+gbjLV2Zqg1QpOwCqmrc83aRFTL2r7J8XL4ibM7PjrDBYvRs9EjbiHZSegLd5JHRm6Yq9ViZpEipJv7Btil7KHgv1B750iwPnweD/AE6DLXkAruX3l1bEJAItdkxyc2YHWApVLn5FLfc3Ou4dfiBkaNxefB2+Ke+FK2RNKZH/U5yCxkohrz5pJMMD/Y371GD9TvCTnUDLqWPJVq7euP5EQ7it9iodQXqakBjDCz7wVKnLe134FcsFgpJ2kiQjiBZ5nZ2HjtCTmAHugdAMwRB3mnRGoSLRGf/2xweaoSroAn4w0zb0lPjYd4LLAg43K9o0H7nLYR9PMfpldAK1TeAMEpqRbkLqRWlNa/41DW5HP9aqx2ZBGdn1ifXpxqbkzHivWX3uryFH9z5Tb+a1gAd0WrKTXEHGJnoqMgePKij5BO6p8Ur28O7Fa2EHdD28RZGuA5kzhqfOOW0GSaDrVA/7LvJNb4QGiiezhcqsy0u4B9qfpOjvR29Cmywp2eGaa/qvYCmhFL9GYBTkf5u55WPBzex17HnNhI8LzX8f/KdzQkz+rwy1ie00Q+kqiFbye/ZEo4gW0LKuAZ+J+0qv6uPwNsf3uoAb9T5DMtAjPj9b37YEVtYl62rxMLtV5xNd3KP5e4TlfYg6QP5VpUFbjPuMUMpgbwHfnK1LRIT2dvej0vAio8NtpRbhP73O4AHAx+IQaTdrIvtJNqKazRX/KP3f7MMq2rxlM13cpEE3g3OC5DiJM9f8cWuR3QdrweZNITwFlqb/kCsg0B0hx90bmNfGx9Y/4Y3pAYIW3R95oD+dfTZdYV9lf1UnwhXhpQEYfbD7NaSegVcRQ4VJnmS8aHxcsyPyAt7fbgGacdMRB/4O+bXioXQxeSC5E1VPfoDHkeNDhUaYyBq2NF9iVoNv4OMYpbLJ2xboJF5DvcCS0g9QPfZA4KTcKzoGgG5ybbRyIsXRGWRMddpjaXA/SN9P7QUeRw9gZtuO2ghvjsyGi5TOsqpORX7GeBEc7JeC3gX4b2FMoX6QvAZmGb4Di9A2ac0Y+7MvPM+0BMoJBeXWTARkBraK96DxyS7cE+tnK8M6mRmFeqJsfF6tl5njn4wcAt50DoLBVE/ouUm3W9z+L5nhbmGjWj/BGu66mUcj21nVK3ldqBNwLjAkjgY6ApVoIQ7GaqedwTeRwdLR4TJ9n3I+vCVz15+ksrRS9N1zXmyZp+Ex3HT4kVsTGqQN0N3IsNTAXxwlADuwnXJVRlNApeoE9l78Bh/7J0nHkDPAfhiVbujPDhlEhEgOHGbbOU7KR4UpP8jbCV1jh6rnInsoHPz1RBJ/BJ6YtoUWIteMl5pi1l8tITQo5SS2okA9BLdBp9TJqJt07ugT7D72dd6muxvVKsRpF1bKE+KdnZ95u1JVWU/t3Th7tnnZbWG0TkoLdC7uUOdc865yIR976tS99ln8efmEN9ZcEkMzrc2+SJUl8NoR/TnXvmL6RL9Q+BjZFUNjf9woOZs4xl0NXwH2oTeqfULygyKP26877UwGgTmilM1RZjnK+zsyGZi64213PNhTPYCXi7WA05Rh9GxwE5bjmwjb2dJAjD10cZACa1ZKASJaVl4jn+DbzC7Gh6ud7MhHQLYr3dCbzGjRN7p3sbuPYm8j3+OERl1GAPz2CruVtgP6KPokx0rfG9+SX1jRiWcziF7c9V2p21QXQzvIf9vvUZWeUuT2xFD2sLlY/Cm8ISa2Hv+27Gh6r/UoMhRVgaGI1WhV5gE9SFjpU4FvsNyxfawl+xX4RqKFv1Mi6f6hLrq14gqsKNVd6mvY2yQ+Tmmb/DUvIIzoIzzMMhId7MVGUgOh6r52vpY6EL4tzgeas6P8Z+mxCivYk2kBu8ZPuoa/Gz0O1Mdeqi2S5MIbC92BgFjMHO0MukGDwhvpPbh6wGViQAJV875vsoJKdX2rOlv4i2Vi1yPlqhvMO8l/YJa0NBcm5mJxvT0fg5xcMEgSbJx/oYX7nlWJvogaTJTbOaBFr5LPQJNJu654CJ5cbr5peeIYm/2d9im7Tz4WqJDkg/JWr5+I3EBqMCmS+Iyp/A7XCZUVN6YHf13/NTaHViANU1pjEjzQ3cYF9EwD1f069CHfDF7muSKf3E/E9bHX/s9HO322MF2r/KmQ2MUrdGDmgNvSPwzspkvVxvplHJoeJNRhfuKPtpC/k+9Ru83FdX62pp0CHTx80Cb3FDkDXwP36VumrkRy29JtczAgUOB1YKn2YL/VRUoMB4hv5Zr5+ord6USsQn4Y6sGxqFcdy2TD3yemK30zDZIJITvGINB8uRQSYc9eLHtCLhMjZDWKs0EY/jDYK1IkW+2uYDrbovk76LCd7lkVQ4h59Ct+ctsKk2hy+VGoQ3Je6ZfmqXdiVEWKL3KZ+Gd3lt+RzxA3aQTtNzgEGZNa7C3QBAzRFOkB/TW7OnnMPqUmiM/Ka1n7rAveF/lDkq5Fpk6D56Wt6GBlKD8VXYenRT5mcKxmYahVJ9sMwpSa+wxTQmT5ej8WKK53igzM0i64hn6PvaRgmwvFgVXIBuzcwP7wudD5boHwBXs0VGNXyPjGTKUqUR0hxCjUWb+na405wOghLdn6gbq2U11BoKUyNTscZsRbZcCmhtcUx60wvCLtYoNdl9mbpKniK3+pbFi8ko5ZEf8Tti+33NpH3qo8SSiBs5pJSFngqj0K+42sYkcQi3FHmudaFuauOCBewS5lOUQAgonD4qTQcKwEvy6OQA9yf/Tayf/Ln43HMcnq92I3cj34bYbJ3wCH88gwo/WxFPUMp1rlH51i9Gn9Rd9DB7EWoZTYYs+hfmbfPVyBSjKjMMs8M5CVfsCe536pl3jPrJ2onLJkttgofJ42PDqdWJ8U4t7gTfIlnfc9+S4V1Ka2FzeIR3p5qbmQx2JB+St1LtEi0h0d+QyeVO0a+ra7R1NuRrDZ7zbzVRbih30LcEmeq9I01mvhL7Mq8BE6QOAT1wgQ0oI63b4rZEH3abUTdwWX6XqoscC8707bRPGXyoDWVBf7HfejR/3fi96EN5RaRHsrrpJQo5g3tsAGI88cCaqTaQpmR+ltMKDDzx1wocjaDaH/H/0j3NiyIl/o3zTJ51hzjKH0dbgncwBrK4hUQ/QCA2Z+oh/4E45LJ2QL7sv4CvorPJA+wsPRyrUrY7tviR1Q16Snq0v7R/QBFX6U8yaxP9wjMz6+PTNcj+CilDV6k/sJix0H3DG45WusPMpnQ40olp7f/aFYU8oQjeHN2Z/BIN6Heyv7tvcv3jQ2xNYoH9ZPXMXecTZCkwA6miCf8ST/fY3+hyYUzqepRlvcqz0AeR+ewvfCka8jz3FWST6Xnuu7hKashJMi3SvIl0EwZBLZnGUDW7e7qO6HiLkEfZO/H2SJ1Msyzi9o++xhcFLuiEUJshw1hyhDGAWRws185nZkfHqvs85+Ur7i3hLWRlrDDsMXtR+7EiuiZIyTVxI5RH9uai3pD0wioEJhqJ0PBsCfpIcszvYpf4Am8b9QRVEizGjlqauVTpxiXpE9F3LApuCjvxc76J3D7o/3UfkvU="


def _load_idx():
    return np.frombuffer(
        zlib.decompress(base64.b64decode(IDX_B64)), dtype="<u2"
    ).astype(np.int64)


def build_nc():
    nc = bacc.Bacc("TRN2", target_bir_lowering=False, debug=False)

    kaug = nc.dram_tensor("kaug", [100, N], BF16, kind="ExternalInput")
    qaug = nc.dram_tensor("qaug", [100, ROWS], BF16, kind="ExternalInput")
    pkaug = nc.dram_tensor("pkaug", [100, SAMPLE], BF16, kind="ExternalInput")
    pqaug = nc.dram_tensor("pqaug", [100, 256], BF16, kind="ExternalInput")
    diagm = nc.dram_tensor("diagm", [128, 16], BF16, kind="ExternalInput")
    lomask = nc.dram_tensor("lomask", [128, NEX], U32, kind="ExternalInput")
    himask = nc.dram_tensor("himask", [128, NEX], U32, kind="ExternalInput")

    knn_out = nc.dram_tensor("knn_out", [128, 2 * NB], F32, kind="ExternalOutput")
    stats_out = nc.dram_tensor("stats_out", [128, 2 * 4 * 5], F32, kind="ExternalOutput")

    W16 = NEX * 16  # 1408

    with tile.TileContext(nc) as tc:
        with ExitStack() as ctx:
            const = ctx.enter_context(tc.tile_pool(name="const", bufs=1))
            big = ctx.enter_context(tc.tile_pool(name="big", bufs=1))
            psum = ctx.enter_context(tc.tile_pool(name="psum", bufs=1, space="PSUM"))
            wp = ctx.enter_context(tc.tile_pool(name="wp", bufs=1))
            wpbig = ctx.enter_context(tc.tile_pool(name="wpbig", bufs=2))
            pp = ctx.enter_context(tc.tile_pool(name="pp", bufs=1))
            outp = ctx.enter_context(tc.tile_pool(name="outp", bufs=1))

            kaug_sb = const.tile_from(kaug.ap(), name="kaug_sb")
            qaug_sb = const.tile_from(qaug.ap(), name="qaug_sb")
            pkaug_sb = const.tile_from(pkaug.ap(), name="pkaug_sb")
            pqaug_sb = const.tile_from(pqaug.ap(), name="pqaug_sb")
            diagm_sb = const.tile_from(diagm.ap(), name="diagm_sb")
            lomask_sb = const.tile_from(lomask.ap(), name="lomask_sb")
            himask_sb = const.tile_from(himask.ap(), name="himask_sb")

            knn_acc = outp.tile([128, 2 * NB], F32, tag="knn_acc")
            stats = outp.tile([128, 2 * 4 * 5], F32, tag="stats")

            # packed arrays (f32; low u16 = column code, high u16 = bf16(-d^2))
            packedH0 = big.tile([128, N // 2], F32, tag="packedH0")
            packedH1 = big.tile([128, N // 2], F32, tag="packedH1")
            pA = big.tile([128, N], F32, tag="pA")
            pB = big.tile([128, N], F32, tag="pB")

            # init the code halves once (even u16 slots)
            for hf, pk in enumerate((packedH0, packedH1)):
                lo = pk[:, :].bitcast(U16).rearrange("p (n two) -> p n two", two=2)
                nc.gpsimd.iota(lo[:, :, 0:1].squeeze(2), pattern=[[1, N // 2]],
                               base=hf * (N // 2), channel_multiplier=0)
            nc.gpsimd.load_library(library_config.ap_gather)

            psC = psum.tile([128, 1536], F32, tag="psC")
            psEa = psum.tile([128, 1024], F32, tag="psEa")
            psEb = psum.tile([128, 1024], F32, tag="psEb")
            psP = psum.tile([128, 512], F32, tag="psP")

            def mm_coord(ps_ap, q_sb, k_sb, qsl, csl):
                nc.tensor.matmul(ps_ap, q_sb[0:16, qsl], k_sb[0:16, csl],
                                 start=True, stop=True)

            def mm_emb(ps_ap, q_sb, k_sb, qsl, csl):
                nc.tensor.matmul(ps_ap, q_sb[32:64, qsl], k_sb[32:64, csl],
                                 start=True, stop=False)
                nc.tensor.matmul(ps_ap, q_sb[64:100, qsl], k_sb[64:100, csl],
                                 start=False, stop=True)

            for b in range(NB):
                qsl = slice(b * 128, (b + 1) * 128)
                p_sb = pA if b % 2 == 0 else pB
                his = [pk[:, :].bitcast(BF16).rearrange(
                    "p (n two) -> p n two", two=2)
                    for pk in (packedH0, packedH1)]

                # ---- coord matmuls + packed move (1536-col groups) ----
                for g in range(8):
                    for c in range(3):
                        csl = slice(g * 1536 + c * 512, g * 1536 + (c + 1) * 512)
                        mm_coord(psC[:, c * 512:(c + 1) * 512], qaug_sb, kaug_sb,
                                 qsl, csl)
                    # value bf16 -> high halves of packed
                    hf, go = (0, g) if g < 4 else (1, g - 4)
                    nc.scalar.activation(
                        his[hf][:, go * 1536:(go + 1) * 1536, 1:2].squeeze(2),
                        psC[:, :], AF.Copy)

                # ---- emb matmuls + sqrt move (1024-col groups) ----
                for g in range(NEH):
                    psE = psEa if g % 2 == 0 else psEb
                    for h in range(2):
                        esl = slice(g * 1024 + h * 512, g * 1024 + (h + 1) * 512)
                        mm_emb(psE[:, h * 512:(h + 1) * 512], qaug_sb, kaug_sb,
                               qsl, esl)
                    nc.scalar.activation(
                        p_sb[:, g * 1024:(g + 1) * 1024], psE[:, :],
                        AF.Sqrt, scale=-1.0)

                # ---- level-1 extraction: top-8 per 384 columns ----
                cand = wp.tile([128, 32 * 8], F32, tag="cand")
                for h in range(32):
                    pk = packedH0 if h < 16 else packedH1
                    ho = h if h < 16 else h - 16
                    nc.vector.max(cand[:, h * 8:(h + 1) * 8],
                                  pk[:, ho * 384:(ho + 1) * 384])

                # ---- level-2: top-88 of the 256 candidates ----
                mvp = wp.tile([128, NEX], F32, tag="mvp")
                work = wp.tile([128, 32 * 8], F32, tag="work")
                nc.vector.max(mvp[:, 0:8], cand[:, :])
                nc.vector.match_replace(out=work[:, :], in_to_replace=mvp[:, 0:8],
                                        in_values=cand[:, :], imm_value=NEG_BIG)
                for r in range(1, 11):
                    nc.vector.max(mvp[:, 8 * r:8 * r + 8], work[:, :])
                    if r < 10:
                        nc.vector.match_replace(
                            out=work[:, :], in_to_replace=mvp[:, 8 * r:8 * r + 8],
                            in_values=work[:, :], imm_value=NEG_BIG)

                # ---- decode: column indices + d^2 values ----
                mvp_u = mvp[:, :].bitcast(U32)
                col32 = wp.tile([128, NEX], U32, tag="col32")
                nc.vector.tensor_tensor(col32[:, :], mvp_u, lomask_sb[:, :],
                                        op=ALU.bitwise_and)
                col16 = wp.tile([128, NEX], mybir.dt.int16, tag="col16")
                nc.vector.tensor_copy(col16[:, :], col32[:, :])
                d2bits = wp.tile([128, NEX], U32, tag="d2bits")
                nc.vector.tensor_tensor(d2bits[:, :], mvp_u, himask_sb[:, :],
                                        op=ALU.bitwise_and)
                d2c = wp.tile([128, NEX], F32, tag="d2c")
                nc.vector.tensor_scalar(d2c[:, :], d2bits[:, :].bitcast(F32),
                                        -1.0, 0.0, op0=ALU.mult, op1=ALU.max)
                d88 = wp.tile([128, NEX], BF16, tag="d88")
                nc.scalar.activation(d88[:, :], d2c[:, :], AF.Sqrt)

                # ---- w = exp(-gamma*d) via polynomial (Horner) ----
                w88 = wp.tile([128, NEX], BF16, tag="w88")
                wtmp = wp.tile([128, NEX], BF16, tag="wtmp")
                cf = [float(v) for v in EXP_COEF]
                deg = len(cf) - 1
                nc.vector.tensor_scalar(w88[:, :], d88[:, :], cf[deg],
                                        cf[deg - 1], op0=ALU.mult, op1=ALU.add)
                for k in range(deg - 2, -1, -1):
                    nc.vector.tensor_tensor(wtmp[:, :], w88[:, :], d88[:, :],
                                            op=ALU.mult)
                    nc.vector.tensor_scalar(w88[:, :], wtmp[:, :], cf[k], None,
                                            op0=ALU.add)
                # slots 86, 87 (ranks beyond self+85) are excluded
                nc.vector.memset(w88[:, SEL:NEX], 0.0)

                # ---- gather pred at selected columns (16-wrapped) ----
                psel = wpbig.tile([128, W16], F32, tag="psel")
                nc.gpsimd.ap_gather(
                    psel[:, :].unsqueeze(2), p_sb[:, :].unsqueeze(2),
                    col16[:, :], channels=128, num_elems=N, d=1, num_idxs=W16)
                # clamp NaN (self column: sqrt of tiny negative) to 0 and
                # downcast so the term chain runs at bf16 DVE rates
                pselb = wpbig.tile([128, W16], BF16, tag="pselb")
                nc.vector.tensor_scalar(pselb[:, :], psel[:, :], 0.0, None,
                                        op0=ALU.max)

                # ---- term: sum (pred - d)^2 * w over diag slots < SEL ----
                psel3 = pselb[:, :].rearrange("p (a b) -> p a b", b=16)
                d3 = d88[:, :].unsqueeze(2).broadcast_to([128, NEX, 16])
                w3 = w88[:, :].unsqueeze(2).broadcast_to([128, NEX, 16])
                wm = wpbig.tile([128, NEX, 16], BF16, tag="wm")
                dg3 = diagm_sb[:, :].unsqueeze(1).broadcast_to([128, NEX, 16])
                nc.vector.tensor_tensor(psel3, psel3, d3, op=ALU.subtract)
                nc.vector.tensor_tensor(wm[:, :, :], w3, dg3, op=ALU.mult)
                nc.vector.tensor_tensor(wm[:, :, :], psel3, wm[:, :, :],
                                        op=ALU.mult)
                nc.vector.tensor_tensor(wm[:, :, :], psel3, wm[:, :, :],
                                        op=ALU.mult)
                nc.vector.tensor_scalar(
                    wm[:, :, :], wm[:, :, :], 1.0, None,
                    op0=ALU.mult, op1=ALU.add,
                    accum_out=knn_acc[:, 2 * b:2 * b + 1])

            # ---- pearson ----
            for qb in range(2):
                qsl = slice(qb * 128, (qb + 1) * 128)
                for p in range(4):
                    psl = slice(p * 500, (p + 1) * 500)
                    col0 = (qb * 4 + p) * 5
                    psc = psP[:, 0:500]
                    cd = pp.tile([128, 500], BF16, tag="cd")
                    ed = pp.tile([128, 500], BF16, tag="ed")
                    pjunk = pp.tile([128, 500], BF16, tag="pjunk")
                    mm_coord(psc, pqaug_sb, pkaug_sb, qsl, psl)
                    # cd = sqrt(d^2), accum sum(cd); sum(d^2) via Copy(-in)
                    nc.scalar.activation(cd[:, :], psc, AF.Sqrt, scale=-1.0,
                                         accum_out=stats[:, col0:col0 + 1])
                    nc.scalar.activation(
                        pjunk[:, :], psc, AF.Copy, scale=-1.0,
                        accum_out=stats[:, col0 + 2:col0 + 3])
                    pse = psP[:, 0:500]
                    mm_emb(pse, pqaug_sb, pkaug_sb, qsl, psl)
                    nc.scalar.activation(ed[:, :], pse, AF.Sqrt, scale=-1.0,
                                         accum_out=stats[:, col0 + 1:col0 + 2])
                    nc.scalar.activation(
                        pjunk[:, :], pse, AF.Copy, scale=-1.0,
                        accum_out=stats[:, col0 + 3:col0 + 4])
                    nc.vector.tensor_tensor(pjunk[:, :], cd[:, :], ed[:, :],
                                            op=ALU.mult)
                    nc.vector.tensor_scalar(
                        pjunk[:, :], pjunk[:, :], 1.0, None,
                        op0=ALU.mult, op1=ALU.add,
                        accum_out=stats[:, col0 + 4:col0 + 5])

            nc.sync.dma_start(knn_out.ap(), knn_acc[:, :])
            nc.sync.dma_start(stats_out.ap(), stats[:, :])

    nc.compile()
    return nc


def _split_bf16(x):
    hi = x.astype(BF)
    lo = (x - hi.astype(np.float32)).astype(BF)
    return hi, lo


def _aug_pair(coords, emb, n):
    """key_aug, query_aug [100, n] bf16 with norms folded (PSUM = -d^2-eps).

    The norms are computed from the ROUNDED (bf16 / hi+lo) values the
    matmul actually multiplies, so the self column lands at ~0 exactly;
    the eps (folded into the key+query norms, half each) keeps PSUM
    strictly negative so the ACT sqrt of -PSUM never sees a negative
    input. It shifts every d^2 uniformly (ranking-invariant) by 4e-4
    (coord) / 1e-2 (emb) - below the bf16 quantisation already
    accepted."""
    ch, cl = _split_bf16(coords.T)  # [3, n]
    chl = ch.astype(np.float32) + cl.astype(np.float32)
    cn = (chl * chl).sum(axis=0) + 4.0e-4
    eh = emb.T.astype(BF)  # [64, n]
    ef = eh.astype(np.float32)
    en = (ef * ef).sum(axis=0) + 1.0e-2
    cnh, cnl = _split_bf16(cn)
    enh, enl = _split_bf16(en)

    k = np.zeros((100, n), BF)
    k[0:3] = (2.0 * ch.astype(np.float32)).astype(BF)
    k[3:6] = (2.0 * cl.astype(np.float32)).astype(BF)
    k[6:9] = k[0:3]
    k[9:12] = k[3:6]
    k[12] = (-cnh.astype(np.float32)).astype(BF)
    k[13] = (-cnl.astype(np.float32)).astype(BF)
    k[14] = BF(1.0)
    k[15] = BF(1.0)
    k[32:96] = (2.0 * eh.astype(np.float32)).astype(BF)
    k[96] = (-enh.astype(np.float32)).astype(BF)
    k[97] = (-enl.astype(np.float32)).astype(BF)
    k[98] = BF(1.0)
    k[99] = BF(1.0)

    q = np.zeros((100, n), BF)
    q[0:3] = ch
    q[3:6] = ch
    q[6:9] = cl
    q[9:12] = cl
    q[12] = BF(1.0)
    q[13] = BF(1.0)
    q[14] = (-cnh.astype(np.float32)).astype(BF)
    q[15] = (-cnl.astype(np.float32)).astype(BF)
    q[32:96] = eh
    q[96] = BF(1.0)
    q[97] = BF(1.0)
    q[98] = (-enh.astype(np.float32)).astype(BF)
    q[99] = (-enl.astype(np.float32)).astype(BF)
    return k, q


def _host_consts():
    q = np.arange(16)
    p = np.arange(128)
    diagm = (q[None, :] == (p[:, None] % 16)).astype(BF)
    lomask = np.full((128, NEX), 0x3FFF, np.uint32)
    himask = np.full((128, NEX), 0xFFFF0000, np.uint32)
    return diagm, lomask, himask


def _make_in_maps(embeddings, coords):
    embeddings = np.ascontiguousarray(embeddings, dtype=np.float32)
    coords = np.ascontiguousarray(coords, dtype=np.float32)
    kaug, qaug = _aug_pair(coords, embeddings, N)

    idx = _load_idx()
    pkaug, pq_full = _aug_pair(coords[idx], embeddings[idx], SAMPLE)
    diagm, lomask, himask = _host_consts()

    sq_per = SAMPLE // NCORES  # 250
    in_maps = []
    for d in range(NCORES):
        r0 = d * ROWS
        q = np.ascontiguousarray(qaug[:, r0:r0 + ROWS])
        pq = np.zeros((100, 256), BF)
        pq[:, :sq_per] = pq_full[:, d * sq_per:(d + 1) * sq_per]
        in_maps.append({
            "kaug": kaug, "qaug": q, "pkaug": pkaug, "pqaug": pq,
            "diagm": diagm,
            "lomask": lomask, "himask": himask,
        })
    return in_maps


def _combine(results):
    knn_sum = 0.0
    s_cd = s_ed = s_cd2 = s_ed2 = s_edcd = 0.0
    for r in results:
        knn_sum += r["knn_out"].astype(np.float64).sum()
        st = r["stats_out"].astype(np.float64).reshape(128, -1, 5)
        s_cd += st[:, :, 0].sum()
        s_ed += st[:, :, 1].sum()
        s_cd2 += st[:, :, 2].sum()
        s_ed2 += st[:, :, 3].sum()
        s_edcd += st[:, :, 4].sum()

    m = float(SAMPLE) * float(SAMPLE)
    e_cd, e_ed = s_cd / m, s_ed / m
    e_cd2, e_ed2, e_edcd = s_cd2 / m, s_ed2 / m, s_edcd / m
    es = np.sqrt(max(e_ed2 - e_ed * e_ed, 0.0) + 1e-8)
    cs = np.sqrt(max(e_cd2 - e_cd * e_cd, 0.0) + 1e-8)
    pearson = (e_edcd - e_ed * e_cd) / (es * cs + 1e-8)
    pearson_loss = 1.0 - pearson

    local_loss = knn_sum / (float(N) * float(KNN))
    return np.float32(pearson_loss + 0.5 * local_loss)


_NC_CACHE = {}


def _get_nc():
    if "v2" not in _NC_CACHE:
        _NC_CACHE["v2"] = build_nc()
    return _NC_CACHE["v2"]


def _run_device(embeddings, coords, trace=False):
    in_maps = _make_in_maps(embeddings, coords)
    nc = _get_nc()
    return run_bass_kernel_spmd(nc, in_maps, core_ids=list(range(NCORES)),
                                trace=trace)


def kernel(embeddings, coords):
    res = _run_device(embeddings, coords, trace=False)
    return _combine(res.results)


# revision 29
# speedup vs baseline: 1.0450x; 1.0450x over previous
"""KNNDistanceLoss Trainium2 Bass kernel (v2).

loss = pearson_loss(2000-sample distance matrices) + 0.5 * local_loss
local_loss = mean over (i, 85 coord-NN of i) of (d_emb - d_coord)^2
             * exp(-gamma * d_coord)

Design (8 cores, each owns N/8 = 1536 query rows; keys replicated):
  - All norms folded into the augmented matmul, so PSUM holds -d^2
    directly (coords + |k|^2 + |q|^2 rows, hi/lo bf16 splits; exact to
    ~2^-16 rel).
  - ACT copies bf16(-d^2_coord) into the HIGH u16 halves of an f32
    "packed" array whose LOW halves hold a column-index constant: each
    packed f32 sorts by distance value and carries its column index.
  - DVE extraction: per-256-column max8 (top-8, prob of losing a true
    top-86 neighbour ~1e-4 per row-half, error per event ~1e-4 of the
    loss) -> 384 candidates; 11 rounds max8+match_replace -> top-88
    packed values = values + column indices. No max_index, no masks.
  - Sparse term phase: decode (bit ops), sqrt on [128,88] tiles,
    exp via DVE polynomial (keeps ACT on the sqrt table set forever),
    GPSIMD indirect_copy gathers pred at the selected columns
    (16-partition-wrapped; each row's own values land at stride-16
    diagonal slots, selected by a static mask), fused
    tensor_tensor_reduce accumulates (pred-true)^2 * w.
  - pred = sqrt(-d^2_emb) is produced straight from PSUM by the ACT
    sqrt move (the move IS the sqrt).
Self distance ~0 ranks first and contributes a zero term, matching the
reference's topk(k+1) drop-self exactly.
"""

import base64
import zlib
from contextlib import ExitStack

import numpy as np
import ml_dtypes

import concourse.bass as bass
import concourse.bacc as bacc
import concourse.mybir as mybir
import concourse.tile as tile
from concourse import library_config
from concourse.bass_utils import run_bass_kernel_spmd

F32 = mybir.dt.float32
BF16 = mybir.dt.bfloat16
U16 = mybir.dt.uint16
U32 = mybir.dt.uint32
AF = mybir.ActivationFunctionType
ALU = mybir.AluOpType

BF = ml_dtypes.bfloat16

N, D, C = 12288, 64, 3
KNN = 85
SEL = KNN + 1  # 86 = self + 85 nearest
NEX = 88      # extracted per row (11 rounds of max8)
GAMMA = 0.5
SAMPLE = 2000
NCORES = 8
ROWS = N // NCORES   # 1536
NB = ROWS // 128     # 12
NCH = N // 2048      # 6 psum-groups per block (coord)
NEH = N // 1024      # 12 psum-groups per block (emb)

NEG_BIG = -1.0e30

# exp(-GAMMA*d) on [0, 10] via degree-8 power-series fit (abs err ~2e-5)
_dg = np.linspace(0.0, 10.0, 4001)
_cheb = np.polynomial.chebyshev.Chebyshev.fit(_dg, np.exp(-GAMMA * _dg), 8)
EXP_COEF = _cheb.convert(kind=np.polynomial.Polynomial).coef  # c0..c8

# jax.random.permutation(jax.random.key(42), 12288)[:2000] -- fixed sample.
IDX_B64 = "eNoFwQdg1IQCANDcXcYluVz25VZyl1zGJZdcEIXC56NsLBakLAvSsgWRKSC7yBCQIXujoEwLZUjtB0FBQGTPyqgs2QKytAgC8t9rhYyFNykYeZct9Xwa0ZPRWI1kD2kX7yHexquwdtpgN58cTQ41Fpuj9LjnmZRGQ66gzfeJ+gt+hVwgtify4frkcP47Y1MyD++Dp/yzYtsFUpjLRYPL5G+ZRfH7RA5UL9hS5oVJqffYu5EG8C6BTe/lroBxe3ESkNqkDwmk0jJ5K/owK7oHuV5qP2khOdboIJXjfcL12I2BF0ROZqaVBT5lV+LD1E7+7xU5dhHel2wr8c4i/TX6hlSV1MKNiLxQT08dqbu1EqprIfYYJxSdG7kF/GGfkBtg4+zrZnHqDnuaCVG3qKHoYHUitDA9PdZYe8XTG3gd6OpFgzWAHzM3UiPd0cIytix71b5mdPKsMz5L7fBfym5XD9ALUqvtvoFPYiuiTzzj0meBU9YEf54/q7dnlwGqQlGs/Y3gj32ObEsWA3v1A1JzJ2s+kSHrgDqGQqQdQCfIz+alHjoEnU1tz85g91B+upf5UE0S1cCxYnH8kGcJ/zy5kulCTonXDP6VNcwSZk3mebA1B5qvcQv5qwkdZN2W1DUxxgN6F6RH6AzTLbjf68/UticCl0PXoOpg10SFZ4u2yAupHnqQhqD3mZPKAvRQdl/iKTRA/hN5BX4ZNuHR1mJ6I/w80iE20fREe6rrxcbmc3IYvQU/CXYWmzlX03M9cfKaWQ//U2qXaqDrbA/nBXNIbEvkSb3CU/UJXMzfwLlu1MFvseeF98KVyICYz2HNof4W9nd4ezHpXkc3J6/BcpTOnMa6R65o7eGmqCdxK3JasaLZTE0+xUyF7pun4GqEpH7NBdwCbBq/O3nXHZRdR/f2TpUGSGP9uvy5NTW+OpKrS77jyh60InkOGe8WJm5YzSE9lmZ/ImpEsp5S82MJ89xXJwcroZ4ozrNJLBhhX6W28zvSI/RCa0TigtyTwGwZTCdf4PuJHPxLd467RulplzGD4guc77njIUZZbw6KnbTC2YhSI9sh8lwuAnZD85O3qRD5LXnULgiNwK4DR7J9oxOZ5XGMDmQ3iL/FG2ffMAWxfuxRtHfoGfGCa8btIE8nshHSuBtfZf2PHKo94IBAT+s896O5FhD5y/Accm94HbJA3xP+J1YQ3aGctDcibwk3fbnA5/iL1LvWf9AF7C38ReZl8kF2Dz+LWpXoI3isPtZ1zaf8g7X3pJKrmHxfB3ID2A2+ZO2NrGdIchzXP7Qb3xR4ht+0RjM/EyPjc2KV0bUSmN2gve5tKLeKHYDGwR0ZLj1LHG81ie+S9ioLTRQbCf9CFyR+Nw7qY1NToJGe8sBd77IUCg62X4k3yPwEDI9+k30rVAkeYcv9/1qvBkc6pdFjbGWsjm8eeBEYgWSxLfAH3rpQ50hx6At2D4N4G5Mmc9oZ6T+lFnAO/h68HPhUv+gbjLV2Zqg1QpOwCqmrc83aRFTL2r7J8XL4ibM7PjrDBYvRs9EjbiHZSegLd5JHRm6Yq9ViZpEipJv7Btil7KHgv1B750iwPnweD/AE6DLXkAruX3l1bEJAItdkxyc2YHWApVLn5FLfc3Ou4dfiBkaNxefB2+Ke+FK2RNKZH/U5yCxkohrz5pJMMD/Y371GD9TvCTnUDLqWPJVq7euP5EQ7it9iodQXqakBjDCz7wVKnLe134FcsFgpJ2kiQjiBZ5nZ2HjtCTmAHugdAMwRB3mnRGoSLRGf/2xweaoSroAn4w0zb0lPjYd4LLAg43K9o0H7nLYR9PMfpldAK1TeAMEpqRbkLqRWlNa/41DW5HP9aqx2ZBGdn1ifXpxqbkzHivWX3uryFH9z5Tb+a1gAd0WrKTXEHGJnoqMgePKij5BO6p8Ur28O7Fa2EHdD28RZGuA5kzhqfOOW0GSaDrVA/7LvJNb4QGiiezhcqsy0u4B9qfpOjvR29Cmywp2eGaa/qvYCmhFL9GYBTkf5u55WPBzex17HnNhI8LzX8f/KdzQkz+rwy1ie00Q+kqiFbye/ZEo4gW0LKuAZ+J+0qv6uPwNsf3uoAb9T5DMtAjPj9b37YEVtYl62rxMLtV5xNd3KP5e4TlfYg6QP5VpUFbjPuMUMpgbwHfnK1LRIT2dvej0vAio8NtpRbhP73O4AHAx+IQaTdrIvtJNqKazRX/KP3f7MMq2rxlM13cpEE3g3OC5DiJM9f8cWuR3QdrweZNITwFlqb/kCsg0B0hx90bmNfGx9Y/4Y3pAYIW3R95oD+dfTZdYV9lf1UnwhXhpQEYfbD7NaSegVcRQ4VJnmS8aHxcsyPyAt7fbgGacdMRB/4O+bXioXQxeSC5E1VPfoDHkeNDhUaYyBq2NF9iVoNv4OMYpbLJ2xboJF5DvcCS0g9QPfZA4KTcKzoGgG5ybbRyIsXRGWRMddpjaXA/SN9P7QUeRw9gZtuO2ghvjsyGi5TOsqpORX7GeBEc7JeC3gX4b2FMoX6QvAZmGb4Di9A2ac0Y+7MvPM+0BMoJBeXWTARkBraK96DxyS7cE+tnK8M6mRmFeqJsfF6tl5njn4wcAt50DoLBVE/ouUm3W9z+L5nhbmGjWj/BGu66mUcj21nVK3ldqBNwLjAkjgY6ApVoIQ7GaqedwTeRwdLR4TJ9n3I+vCVz15+ksrRS9N1zXmyZp+Ex3HT4kVsTGqQN0N3IsNTAXxwlADuwnXJVRlNApeoE9l78Bh/7J0nHkDPAfhiVbujPDhlEhEgOHGbbOU7KR4UpP8jbCV1jh6rnInsoHPz1RBJ/BJ6YtoUWIteMl5pi1l8tITQo5SS2okA9BLdBp9TJqJt07ugT7D72dd6muxvVKsRpF1bKE+KdnZ95u1JVWU/t3Th7tnnZbWG0TkoLdC7uUOdc865yIR976tS99ln8efmEN9ZcEkMzrc2+SJUl8NoR/TnXvmL6RL9Q+BjZFUNjf9woOZs4xl0NXwH2oTeqfULygyKP26877UwGgTmilM1RZjnK+zsyGZi64213PNhTPYCXi7WA05Rh9GxwE5bjmwjb2dJAjD10cZACa1ZKASJaVl4jn+DbzC7Gh6ud7MhHQLYr3dCbzGjRN7p3sbuPYm8j3+OERl1GAPz2CruVtgP6KPokx0rfG9+SX1jRiWcziF7c9V2p21QXQzvIf9vvUZWeUuT2xFD2sLlY/Cm8ISa2Hv+27Gh6r/UoMhRVgaGI1WhV5gE9SFjpU4FvsNyxfawl+xX4RqKFv1Mi6f6hLrq14gqsKNVd6mvY2yQ+Tmmb/DUvIIzoIzzMMhId7MVGUgOh6r52vpY6EL4tzgeas6P8Z+mxCivYk2kBu8ZPuoa/Gz0O1Mdeqi2S5MIbC92BgFjMHO0MukGDwhvpPbh6wGViQAJV875vsoJKdX2rOlv4i2Vi1yPlqhvMO8l/YJa0NBcm5mJxvT0fg5xcMEgSbJx/oYX7nlWJvogaTJTbOaBFr5LPQJNJu654CJ5cbr5peeIYm/2d9im7Tz4WqJDkg/JWr5+I3EBqMCmS+Iyp/A7XCZUVN6YHf13/NTaHViANU1pjEjzQ3cYF9EwD1f069CHfDF7muSKf3E/E9bHX/s9HO322MF2r/KmQ2MUrdGDmgNvSPwzspkvVxvplHJoeJNRhfuKPtpC/k+9Ru83FdX62pp0CHTx80Cb3FDkDXwP36VumrkRy29JtczAgUOB1YKn2YL/VRUoMB4hv5Zr5+ord6USsQn4Y6sGxqFcdy2TD3yemK30zDZIJITvGINB8uRQSYc9eLHtCLhMjZDWKs0EY/jDYK1IkW+2uYDrbovk76LCd7lkVQ4h59Ct+ctsKk2hy+VGoQ3Je6ZfmqXdiVEWKL3KZ+Gd3lt+RzxA3aQTtNzgEGZNa7C3QBAzRFOkB/TW7OnnMPqUmiM/Ka1n7rAveF/lDkq5Fpk6D56Wt6GBlKD8VXYenRT5mcKxmYahVJ9sMwpSa+wxTQmT5ej8WKK53igzM0i64hn6PvaRgmwvFgVXIBuzcwP7wudD5boHwBXs0VGNXyPjGTKUqUR0hxCjUWb+na405wOghLdn6gbq2U11BoKUyNTscZsRbZcCmhtcUx60wvCLtYoNdl9mbpKniK3+pbFi8ko5ZEf8Tti+33NpH3qo8SSiBs5pJSFngqj0K+42sYkcQi3FHmudaFuauOCBewS5lOUQAgonD4qTQcKwEvy6OQA9yf/Tayf/Ln43HMcnq92I3cj34bYbJ3wCH88gwo/WxFPUMp1rlH51i9Gn9Rd9DB7EWoZTYYs+hfmbfPVyBSjKjMMs8M5CVfsCe536pl3jPrJ2onLJkttgofJ42PDqdWJ8U4t7gTfIlnfc9+S4V1Ka2FzeIR3p5qbmQx2JB+St1LtEi0h0d+QyeVO0a+ra7R1NuRrDZ7zbzVRbih30LcEmeq9I01mvhL7Mq8BE6QOAT1wgQ0oI63b4rZEH3abUTdwWX6XqoscC8707bRPGXyoDWVBf7HfejR/3fi96EN5RaRHsrrpJQo5g3tsAGI88cCaqTaQpmR+ltMKDDzx1wocjaDaH/H/0j3NiyIl/o3zTJ51hzjKH0dbgncwBrK4hUQ/QCA2Z+oh/4E45LJ2QL7sv4CvorPJA+wsPRyrUrY7tviR1Q16Snq0v7R/QBFX6U8yaxP9wjMz6+PTNcj+CilDV6k/sJix0H3DG45WusPMpnQ40olp7f/aFYU8oQjeHN2Z/BIN6Heyv7tvcv3jQ2xNYoH9ZPXMXecTZCkwA6miCf8ST/fY3+hyYUzqepRlvcqz0AeR+ewvfCka8jz3FWST6Xnuu7hKashJMi3SvIl0EwZBLZnGUDW7e7qO6HiLkEfZO/H2SJ1Msyzi9o++xhcFLuiEUJshw1hyhDGAWRws185nZkfHqvs85+Ur7i3hLWRlrDDsMXtR+7EiuiZIyTVxI5RH9uai3pD0wioEJhqJ0PBsCfpIcszvYpf4Am8b9QRVEizGjlqauVTpxiXpE9F3LApuCjvxc76J3D7o/3UfkvU="


def _load_idx():
    return np.frombuffer(
        zlib.decompress(base64.b64decode(IDX_B64)), dtype="<u2"
    ).astype(np.int64)


def build_nc():
    nc = bacc.Bacc("TRN2", target_bir_lowering=False, debug=False)

    kaug = nc.dram_tensor("kaug", [100, N], BF16, kind="ExternalInput")
    qaug = nc.dram_tensor("qaug", [100, ROWS], BF16, kind="ExternalInput")
    pkaug = nc.dram_tensor("pkaug", [100, SAMPLE], BF16, kind="ExternalInput")
    pqaug = nc.dram_tensor("pqaug", [100, 256], BF16, kind="ExternalInput")
    diagm = nc.dram_tensor("diagm", [128, 16], BF16, kind="ExternalInput")
    lomask = nc.dram_tensor("lomask", [128, NEX], U32, kind="ExternalInput")
    himask = nc.dram_tensor("himask", [128, NEX], U32, kind="ExternalInput")

    knn_out = nc.dram_tensor("knn_out", [128, 2 * NB], F32, kind="ExternalOutput")
    stats_out = nc.dram_tensor("stats_out", [128, 2 * 4 * 5], F32, kind="ExternalOutput")

    W16 = NEX * 16  # 1408

    with tile.TileContext(nc) as tc:
        with ExitStack() as ctx:
            const = ctx.enter_context(tc.tile_pool(name="const", bufs=1))
            big = ctx.enter_context(tc.tile_pool(name="big", bufs=1))
            psum = ctx.enter_context(tc.tile_pool(name="psum", bufs=1, space="PSUM"))
            wp = ctx.enter_context(tc.tile_pool(name="wp", bufs=1))
            wpbig = ctx.enter_context(tc.tile_pool(name="wpbig", bufs=2))
            pp = ctx.enter_context(tc.tile_pool(name="pp", bufs=1))
            outp = ctx.enter_context(tc.tile_pool(name="outp", bufs=1))

            kaug_sb = const.tile_from(kaug.ap(), name="kaug_sb")
            qaug_sb = const.tile_from(qaug.ap(), name="qaug_sb")
            pkaug_sb = const.tile_from(pkaug.ap(), name="pkaug_sb")
            pqaug_sb = const.tile_from(pqaug.ap(), name="pqaug_sb")
            diagm_sb = const.tile_from(diagm.ap(), name="diagm_sb")
            lomask_sb = const.tile_from(lomask.ap(), name="lomask_sb")
            himask_sb = const.tile_from(himask.ap(), name="himask_sb")

            knn_acc = outp.tile([128, 2 * NB], F32, tag="knn_acc")
            stats = outp.tile([128, 2 * 4 * 5], F32, tag="stats")

            # packed arrays (f32; low u16 = column code, high u16 = bf16(-d^2))
            packedH0 = big.tile([128, N // 2], F32, tag="packedH0")
            packedH1 = big.tile([128, N // 2], F32, tag="packedH1")
            pA = big.tile([128, N], F32, tag="pA")
            pB = big.tile([128, N], F32, tag="pB")

            # init the code halves once (even u16 slots)
            for hf, pk in enumerate((packedH0, packedH1)):
                lo = pk[:, :].bitcast(U16).rearrange("p (n two) -> p n two", two=2)
                nc.gpsimd.iota(lo[:, :, 0:1].squeeze(2), pattern=[[1, N // 2]],
                               base=hf * (N // 2), channel_multiplier=0)
            nc.gpsimd.load_library(library_config.ap_gather)

            psC = psum.tile([128, 1536], F32, tag="psC")
            psEa = psum.tile([128, 1024], F32, tag="psEa")
            psEb = psum.tile([128, 1024], F32, tag="psEb")
            psP = psum.tile([128, 512], F32, tag="psP")

            def mm_coord(ps_ap, q_sb, k_sb, qsl, csl):
                nc.tensor.matmul(ps_ap, q_sb[0:16, qsl], k_sb[0:16, csl],
                                 start=True, stop=True)

            def mm_emb(ps_ap, q_sb, k_sb, qsl, csl):
                nc.tensor.matmul(ps_ap, q_sb[32:64, qsl], k_sb[32:64, csl],
                                 start=True, stop=False)
                nc.tensor.matmul(ps_ap, q_sb[64:100, qsl], k_sb[64:100, csl],
                                 start=False, stop=True)

            for b in range(NB):
                qsl = slice(b * 128, (b + 1) * 128)
                p_sb = pA if b % 2 == 0 else pB
                his = [pk[:, :].bitcast(BF16).rearrange(
                    "p (n two) -> p n two", two=2)
                    for pk in (packedH0, packedH1)]

                # ---- coord matmuls + packed move (1536-col groups) ----
                for g in range(8):
                    for c in range(3):
                        csl = slice(g * 1536 + c * 512, g * 1536 + (c + 1) * 512)
                        mm_coord(psC[:, c * 512:(c + 1) * 512], qaug_sb, kaug_sb,
                                 qsl, csl)
                    # value bf16 -> high halves of packed
                    hf, go = (0, g) if g < 4 else (1, g - 4)
                    nc.scalar.activation(
                        his[hf][:, go * 1536:(go + 1) * 1536, 1:2].squeeze(2),
                        psC[:, :], AF.Copy)

                # ---- emb matmuls + sqrt move (1024-col groups) ----
                for g in range(NEH):
                    psE = psEa if g % 2 == 0 else psEb
                    for h in range(2):
                        esl = slice(g * 1024 + h * 512, g * 1024 + (h + 1) * 512)
                        mm_emb(psE[:, h * 512:(h + 1) * 512], qaug_sb, kaug_sb,
                               qsl, esl)
                    nc.scalar.activation(
                        p_sb[:, g * 1024:(g + 1) * 1024], psE[:, :],
                        AF.Sqrt, scale=-1.0)

                # ---- level-1 extraction: top-8 per 512 columns ----
                cand = wp.tile([128, 24 * 8], F32, tag="cand")
                for h in range(24):
                    pk = packedH0 if h < 12 else packedH1
                    ho = h if h < 12 else h - 12
                    nc.vector.max(cand[:, h * 8:(h + 1) * 8],
                                  pk[:, ho * 512:(ho + 1) * 512])

                # ---- level-2: top-88 of the 192 candidates ----
                mvp = wp.tile([128, NEX], F32, tag="mvp")
                work = wp.tile([128, 24 * 8], F32, tag="work")
                nc.vector.max(mvp[:, 0:8], cand[:, :])
                nc.vector.match_replace(out=work[:, :], in_to_replace=mvp[:, 0:8],
                                        in_values=cand[:, :], imm_value=NEG_BIG)
                for r in range(1, 11):
                    nc.vector.max(mvp[:, 8 * r:8 * r + 8], work[:, :])
                    if r < 10:
                        nc.vector.match_replace(
                            out=work[:, :], in_to_replace=mvp[:, 8 * r:8 * r + 8],
                            in_values=work[:, :], imm_value=NEG_BIG)

                # ---- decode: column indices + d^2 values ----
                mvp_u = mvp[:, :].bitcast(U32)
                col32 = wp.tile([128, NEX], U32, tag="col32")
                nc.vector.tensor_tensor(col32[:, :], mvp_u, lomask_sb[:, :],
                                        op=ALU.bitwise_and)
                col16 = wp.tile([128, NEX], mybir.dt.int16, tag="col16")
                nc.vector.tensor_copy(col16[:, :], col32[:, :])
                d2bits = wp.tile([128, NEX], U32, tag="d2bits")
                nc.vector.tensor_tensor(d2bits[:, :], mvp_u, himask_sb[:, :],
                                        op=ALU.bitwise_and)
                d2c = wp.tile([128, NEX], F32, tag="d2c")
                nc.vector.tensor_scalar(d2c[:, :], d2bits[:, :].bitcast(F32),
                                        -1.0, 0.0, op0=ALU.mult, op1=ALU.max)
                d88 = wp.tile([128, NEX], BF16, tag="d88")
                nc.scalar.activation(d88[:, :], d2c[:, :], AF.Sqrt)

                # ---- w = exp(-gamma*d) via polynomial (Horner) ----
                w88 = wp.tile([128, NEX], BF16, tag="w88")
                wtmp = wp.tile([128, NEX], BF16, tag="wtmp")
                cf = [float(v) for v in EXP_COEF]
                deg = len(cf) - 1
                nc.vector.tensor_scalar(w88[:, :], d88[:, :], cf[deg],
                                        cf[deg - 1], op0=ALU.mult, op1=ALU.add)
                for k in range(deg - 2, -1, -1):
                    nc.vector.tensor_tensor(wtmp[:, :], w88[:, :], d88[:, :],
                                            op=ALU.mult)
                    nc.vector.tensor_scalar(w88[:, :], wtmp[:, :], cf[k], None,
                                            op0=ALU.add)
                # slots 86, 87 (ranks beyond self+85) are excluded
                nc.vector.memset(w88[:, SEL:NEX], 0.0)

                # ---- gather pred at selected columns (16-wrapped) ----
                psel = wpbig.tile([128, W16], F32, tag="psel")
                nc.gpsimd.ap_gather(
                    psel[:, :].unsqueeze(2), p_sb[:, :].unsqueeze(2),
                    col16[:, :], channels=128, num_elems=N, d=1, num_idxs=W16)
                # clamp (max with 0) + downcast so the term chain runs at
                # bf16 DVE rates
                pselb = wpbig.tile([128, W16], BF16, tag="pselb")
                nc.vector.tensor_scalar(pselb[:, :], psel[:, :], 0.0, None,
                                        op0=ALU.max)

                # ---- term: sum (pred - d)^2 * w over diag slots < SEL ----
                psel3 = pselb[:, :].rearrange("p (a b) -> p a b", b=16)
                d3 = d88[:, :].unsqueeze(2).broadcast_to([128, NEX, 16])
                w3 = w88[:, :].unsqueeze(2).broadcast_to([128, NEX, 16])
                wm = wpbig.tile([128, NEX, 16], BF16, tag="wm")
                dg3 = diagm_sb[:, :].unsqueeze(1).broadcast_to([128, NEX, 16])
                nc.vector.tensor_tensor(psel3, psel3, d3, op=ALU.subtract)
                nc.vector.tensor_tensor(wm[:, :, :], w3, dg3, op=ALU.mult)
                nc.vector.tensor_tensor(wm[:, :, :], psel3, wm[:, :, :],
                                        op=ALU.mult)
                nc.vector.tensor_tensor(wm[:, :, :], psel3, wm[:, :, :],
                                        op=ALU.mult)
                nc.vector.tensor_scalar(
                    wm[:, :, :], wm[:, :, :], 1.0, None,
                    op0=ALU.mult, op1=ALU.add,
                    accum_out=knn_acc[:, 2 * b:2 * b + 1])

            # ---- pearson ----
            for qb in range(2):
                qsl = slice(qb * 128, (qb + 1) * 128)
                for p in range(4):
                    psl = slice(p * 500, (p + 1) * 500)
                    col0 = (qb * 4 + p) * 5
                    psc = psP[:, 0:500]
                    cd = pp.tile([128, 500], BF16, tag="cd")
                    ed = pp.tile([128, 500], BF16, tag="ed")
                    pjunk = pp.tile([128, 500], BF16, tag="pjunk")
                    mm_coord(psc, pqaug_sb, pkaug_sb, qsl, psl)
                    # cd = sqrt(d^2), accum sum(cd); sum(d^2) via Copy(-in)
                    nc.scalar.activation(cd[:, :], psc, AF.Sqrt, scale=-1.0,
                                         accum_out=stats[:, col0:col0 + 1])
                    nc.scalar.activation(
                        pjunk[:, :], psc, AF.Copy, scale=-1.0,
                        accum_out=stats[:, col0 + 2:col0 + 3])
                    pse = psP[:, 0:500]
                    mm_emb(pse, pqaug_sb, pkaug_sb, qsl, psl)
                    nc.scalar.activation(ed[:, :], pse, AF.Sqrt, scale=-1.0,
                                         accum_out=stats[:, col0 + 1:col0 + 2])
                    nc.scalar.activation(
                        pjunk[:, :], pse, AF.Copy, scale=-1.0,
                        accum_out=stats[:, col0 + 3:col0 + 4])
                    nc.vector.tensor_tensor(pjunk[:, :], cd[:, :], ed[:, :],
                                            op=ALU.mult)
                    nc.vector.tensor_scalar(
                        pjunk[:, :], pjunk[:, :], 1.0, None,
                        op0=ALU.mult, op1=ALU.add,
                        accum_out=stats[:, col0 + 4:col0 + 5])

            nc.sync.dma_start(knn_out.ap(), knn_acc[:, :])
            nc.sync.dma_start(stats_out.ap(), stats[:, :])

    nc.compile()
    return nc


def _split_bf16(x):
    hi = x.astype(BF)
    lo = (x - hi.astype(np.float32)).astype(BF)
    return hi, lo


def _aug_pair(coords, emb, n):
    """key_aug, query_aug [100, n] bf16 with norms folded (PSUM = -d^2-eps).

    The norms are computed from the ROUNDED (bf16 / hi+lo) values the
    matmul actually multiplies, so the self column lands at ~0 exactly;
    the eps (folded into the key+query norms, half each) keeps PSUM
    strictly negative so the ACT sqrt of -PSUM never sees a negative
    input. It shifts every d^2 uniformly (ranking-invariant) by 4e-4
    (coord) / 1e-2 (emb) - below the bf16 quantisation already
    accepted."""
    ch, cl = _split_bf16(coords.T)  # [3, n]
    chl = ch.astype(np.float32) + cl.astype(np.float32)
    cn = (chl * chl).sum(axis=0) + 4.0e-4
    eh = emb.T.astype(BF)  # [64, n]
    ef = eh.astype(np.float32)
    en = (ef * ef).sum(axis=0) + 1.0e-2
    cnh, cnl = _split_bf16(cn)
    enh, enl = _split_bf16(en)

    k = np.zeros((100, n), BF)
    k[0:3] = (2.0 * ch.astype(np.float32)).astype(BF)
    k[3:6] = (2.0 * cl.astype(np.float32)).astype(BF)
    k[6:9] = k[0:3]
    k[9:12] = k[3:6]
    k[12] = (-cnh.astype(np.float32)).astype(BF)
    k[13] = (-cnl.astype(np.float32)).astype(BF)
    k[14] = BF(1.0)
    k[15] = BF(1.0)
    k[32:96] = (2.0 * eh.astype(np.float32)).astype(BF)
    k[96] = (-enh.astype(np.float32)).astype(BF)
    k[97] = (-enl.astype(np.float32)).astype(BF)
    k[98] = BF(1.0)
    k[99] = BF(1.0)

    q = np.zeros((100, n), BF)
    q[0:3] = ch
    q[3:6] = ch
    q[6:9] = cl
    q[9:12] = cl
    q[12] = BF(1.0)
    q[13] = BF(1.0)
    q[14] = (-cnh.astype(np.float32)).astype(BF)
    q[15] = (-cnl.astype(np.float32)).astype(BF)
    q[32:96] = eh
    q[96] = BF(1.0)
    q[97] = BF(1.0)
    q[98] = (-enh.astype(np.float32)).astype(BF)
    q[99] = (-enl.astype(np.float32)).astype(BF)
    return k, q


def _host_consts():
    q = np.arange(16)
    p = np.arange(128)
    diagm = (q[None, :] == (p[:, None] % 16)).astype(BF)
    lomask = np.full((128, NEX), 0x3FFF, np.uint32)
    himask = np.full((128, NEX), 0xFFFF0000, np.uint32)
    return diagm, lomask, himask


def _make_in_maps(embeddings, coords):
    embeddings = np.ascontiguousarray(embeddings, dtype=np.float32)
    coords = np.ascontiguousarray(coords, dtype=np.float32)
    kaug, qaug = _aug_pair(coords, embeddings, N)

    idx = _load_idx()
    pkaug, pq_full = _aug_pair(coords[idx], embeddings[idx], SAMPLE)
    diagm, lomask, himask = _host_consts()

    sq_per = SAMPLE // NCORES  # 250
    in_maps = []
    for d in range(NCORES):
        r0 = d * ROWS
        q = np.ascontiguousarray(qaug[:, r0:r0 + ROWS])
        pq = np.zeros((100, 256), BF)
        pq[:, :sq_per] = pq_full[:, d * sq_per:(d + 1) * sq_per]
        in_maps.append({
            "kaug": kaug, "qaug": q, "pkaug": pkaug, "pqaug": pq,
            "diagm": diagm,
            "lomask": lomask, "himask": himask,
        })
    return in_maps


def _combine(results):
    knn_sum = 0.0
    s_cd = s_ed = s_cd2 = s_ed2 = s_edcd = 0.0
    for r in results:
        knn_sum += r["knn_out"].astype(np.float64).sum()
        st = r["stats_out"].astype(np.float64).reshape(128, -1, 5)
        s_cd += st[:, :, 0].sum()
        s_ed += st[:, :, 1].sum()
        s_cd2 += st[:, :, 2].sum()
        s_ed2 += st[:, :, 3].sum()
        s_edcd += st[:, :, 4].sum()

    m = float(SAMPLE) * float(SAMPLE)
    e_cd, e_ed = s_cd / m, s_ed / m
    e_cd2, e_ed2, e_edcd = s_cd2 / m, s_ed2 / m, s_edcd / m
    es = np.sqrt(max(e_ed2 - e_ed * e_ed, 0.0) + 1e-8)
    cs = np.sqrt(max(e_cd2 - e_cd * e_cd, 0.0) + 1e-8)
    pearson = (e_edcd - e_ed * e_cd) / (es * cs + 1e-8)
    pearson_loss = 1.0 - pearson

    local_loss = knn_sum / (float(N) * float(KNN))
    return np.float32(pearson_loss + 0.5 * local_loss)


_NC_CACHE = {}


def _get_nc():
    if "v2" not in _NC_CACHE:
        _NC_CACHE["v2"] = build_nc()
    return _NC_CACHE["v2"]


def _run_device(embeddings, coords, trace=False):
    in_maps = _make_in_maps(embeddings, coords)
    nc = _get_nc()
    return run_bass_kernel_spmd(nc, in_maps, core_ids=list(range(NCORES)),
                                trace=trace)


def kernel(embeddings, coords):
    res = _run_device(embeddings, coords, trace=False)
    return _combine(res.results)
